# revision 9
# baseline (speedup 1.0000x reference)
"""Trainium2 Bass kernel for CriterionMiniBatchCrossImagePair.

Computes: prep = L2norm_C(avgpool4x4(x)) per image -> all BxB pairwise
[N,N] similarity maps for S and T -> KL(softmax_T || softmax_S) batchmean.

Sharding: 8 cores. Each core preps ONE of the 8 images (4 S + 4 T),
AllGathers the prepped bf16 features [256,1024], then computes 2 of the
16 (i,j) pairs (transpose-pairing so each core touches only 2 image
indices). Scalar partials are summed on the host.

Math used per row-block (row softmax over m):
  KL_row = sum_m p_t*(log p_t - log p_s)
         = (1/T) * (sum_m eT*rawT - sum_m eT*rawS) / Z_T - ln Z_T + ln Z_S
with eX = exp(rawX/T), Z_X = sum_m eX. No max-subtraction needed:
raw in [-1,1] so raw/T in [-10,10].
"""

import numpy as np

import concourse.bass as bass
import concourse.mybir as mybir
import concourse.tile as tile
from concourse.bass_utils import run_bass_kernel_spmd

F32 = mybir.dt.float32
BF16 = mybir.dt.bfloat16
I32 = mybir.dt.int32
AF = mybir.ActivationFunctionType
ALU = mybir.AluOpType

TEMPERATURE = 0.1
B, C, H, W = 4, 256, 128, 128
PATCH = 4
PH, PW = H // PATCH, W // PATCH  # 32 x 32
N = PH * PW  # 1024
NCORES = 8
CC = C // 128  # channel chunks of 128
FB = 4  # h-row chunks per channel chunk (32 h rows each)
HROWS = H // FB  # 32
NBLK = N // 128  # 8 row blocks per pair
NPAIR = 2  # pairs per core

# core -> [(n0, m0), (n1, m1)] image-index pairs (covers all 16 (i,j))
PAIRS_PER_CORE = [
    [(0, 0), (1, 1)],
    [(2, 2), (3, 3)],
    [(0, 1), (1, 0)],
    [(2, 3), (3, 2)],
    [(0, 2), (2, 0)],
    [(1, 3), (3, 1)],
    [(0, 3), (3, 0)],
    [(1, 2), (2, 1)],
]


def legalize_waits(nc):
    """Split multi-wait instructions into single-wait NoOps.

    The walrus build in this environment encodes at most one sync-wait per
    instruction (and none on register-offset pseudo DMAs): anything more dies
    in codegen with "Too many sync wait commands". Semantically, hoisting a
    wait onto a NoOp immediately before the instruction on the same engine
    stream is identical (both block the engine's sequencer).
    """
    n_id = 0
    for f in nc.m.functions:
        for b in f.blocks:
            lst = b.instructions
            out = []
            changed = False
            for ins in lst:
                si = ins.sync_info
                waits = list(si.on_wait) if si and si.on_wait else []
                keep = 0 if isinstance(ins, mybir.InstDMACopy) else 1
                if len(waits) > keep:
                    moved, kept = waits[: len(waits) - keep], waits[len(waits) - keep :]
                    for w in moved:
                        nop = mybir.InstNoOp(name=f"waitnop_{n_id}")
                        n_id += 1
                        nop.engine = ins.engine
                        nop.sync_info = mybir.SyncInfo(on_wait=[w], on_update=[])
                        out.append(nop)
                    ins.sync_info = mybir.SyncInfo(
                        on_wait=kept, on_update=list(si.on_update)
                    )
                    changed = True
                out.append(ins)
            if changed:
                b.instructions = out
    return nc


def build_bass():
    nc = bass.Bass(num_devices=NCORES)

    img = nc.declare_dram_parameter("img", [C, H, W], F32, isOutput=False)
    sel = nc.declare_dram_parameter("sel", [1, 8], I32, isOutput=False)
    out_partial = nc.declare_dram_parameter("out_partial", [1, 1], F32, isOutput=True)

    with tile.TileContext(nc) as tc:
        with (
            tc.tile_pool(name="dram", bufs=1, space="DRAM") as dpool,
            tc.tile_pool(name="consts", bufs=1) as cpool,
        ):
            ag_in = dpool.tile([128, CC, N], BF16, name="ag_in")
            ag_out = dpool.tile(
                [NCORES, 128, CC, N], BF16, addr_space="Shared", name="ag_out"
            )
            ones_col = cpool.tile([128, 1], F32)
            nc.vector.memset(ones_col[:], 1.0)
            ones_row = cpool.tile([1, 128], F32)
            nc.vector.memset(ones_row[:], 1.0)

            # ---------------- Stage A: prep own image ----------------
            with (
                tc.tile_pool(name="prep", bufs=3) as ppool,
                tc.tile_pool(name="prep_ps", bufs=1, space="PSUM") as pspool,
                tc.tile_pool(name="prep_keep", bufs=1) as kpool,
            ):
                u = kpool.tile([128, CC, PH, PW], F32)  # pooled (unnormalized)
                ss_ps = pspool.tile([1, N], F32)  # sum_c u^2
                for cc in range(CC):
                    for fb in range(FB):
                        raw = ppool.tile([128, HROWS, W], F32, tag="raw")
                        nc.sync.dma_start(
                            raw[:],
                            img[cc * 128 : (cc + 1) * 128, fb * HROWS : (fb + 1) * HROWS, :],
                        )
                        wp1 = ppool.tile([128, HROWS, PW], F32, tag="wp1")
                        wp2 = ppool.tile([128, HROWS, PW], F32, tag="wp2")
                        wp = ppool.tile([128, HROWS, PW], F32, tag="wp")
                        nc.vector.tensor_add(wp1[:], raw[:, :, 0::4], raw[:, :, 1::4])
                        nc.vector.tensor_add(wp2[:], raw[:, :, 2::4], raw[:, :, 3::4])
                        nc.vector.tensor_add(wp[:], wp1[:], wp2[:])
                        hp1 = ppool.tile([128, HROWS // 4, PW], F32, tag="hp1")
                        hp2 = ppool.tile([128, HROWS // 4, PW], F32, tag="hp2")
                        nc.vector.tensor_add(hp1[:], wp[:, 0::4, :], wp[:, 1::4, :])
                        nc.vector.tensor_add(hp2[:], wp[:, 2::4, :], wp[:, 3::4, :])
                        nc.vector.tensor_add(
                            u[:, cc, fb * (HROWS // 4) : (fb + 1) * (HROWS // 4), :],
                            hp1[:],
                            hp2[:],
                        )

                # sum over channels of u^2 (via ones-matmul), both c-chunks
                for cc in range(CC):
                    sq = ppool.tile([128, N], F32, tag="sq")
                    ucc = u[:, cc].rearrange("p a b -> p (a b)")
                    nc.vector.tensor_mul(sq[:], ucc, ucc)
                    for h in range(2):
                        nc.tensor.matmul(
                            ss_ps[:, h * 512 : (h + 1) * 512],
                            ones_col[:],
                            sq[:, h * 512 : (h + 1) * 512],
                            start=(cc == 0),
                            stop=(cc == CC - 1),
                        )

                # inv = ss^(-1/2) = exp(-0.5*ln(ss)) on 1 partition
                lnss = kpool.tile([1, N], F32)
                nc.scalar.activation(lnss[:], ss_ps[:], AF.Ln)
                inv = kpool.tile([1, N], F32)
                nc.scalar.activation(inv[:], lnss[:], AF.Exp, scale=-0.5)

                # broadcast inv to 128 partitions via ones-matmul
                inv_b = pspool.tile([128, N], F32)
                for h in range(2):
                    nc.tensor.matmul(
                        inv_b[:, h * 512 : (h + 1) * 512],
                        ones_row[:],
                        inv[:, h * 512 : (h + 1) * 512],
                        start=True,
                        stop=True,
                    )

                feat = kpool.tile([128, CC, N], BF16)
                for cc in range(CC):
                    ucc = u[:, cc].rearrange("p a b -> p (a b)")
                    nc.vector.tensor_mul(feat[:, cc], ucc, inv_b[:])
                    nc.sync.dma_start(ag_in[:, cc, :], feat[:, cc])

            # ---------------- AllGather prepped features ----------------
            nc.gpsimd.collective_compute(
                "AllGather",
                ALU.bypass,
                replica_groups=[list(range(NCORES))],
                ins=[ag_in.opt()],
                outs=[ag_out.opt()],
            )

            # ---------------- Stage B: 2 pairs of similarity maps ----------------
            with (
                tc.tile_pool(name="slots", bufs=1) as spool,
                tc.tile_pool(name="acc", bufs=1) as apool,
                tc.tile_pool(name="work", bufs=2) as wpool,
            ):
                sel_sb = apool.tile([1, 8], I32)
                nc.sync.dma_start(sel_sb[:], sel[:])

                # slot order: NS0, MS0, NS1, MS1, NT0, MT0, NT1, MT1
                slots = []
                for s in range(8):
                    slot = spool.tile([128, CC, N], BF16, name=f"slot{s}")
                    v = nc.sync.value_load(sel_sb[0:1, s : s + 1])
                    nc.sync.dma_start(slot[:], ag_out[bass.ds(v, 1)].squeeze(0))
                    slots.append(slot)

                zT = apool.tile([128, NPAIR * NBLK], F32)
                zS = apool.tile([128, NPAIR * NBLK], F32)
                r1 = apool.tile([128, NPAIR * NBLK], F32)
                r2 = apool.tile([128, NPAIR * NBLK], F32)

                with tc.tile_pool(name="sims_ps", bufs=2, space="PSUM") as simspool:
                    for p in range(NPAIR):
                        ns, ms = slots[2 * p], slots[2 * p + 1]
                        nt, mt = slots[4 + 2 * p], slots[4 + 2 * p + 1]
                        for blk in range(NBLK):
                            col = p * NBLK + blk
                            psS = simspool.tile([128, N], F32, tag="psS")
                            psT = simspool.tile([128, N], F32, tag="psT")
                            for h in range(2):
                                for cc in range(CC):
                                    nc.tensor.matmul(
                                        psT[:, h * 512 : (h + 1) * 512],
                                        nt[:, cc, blk * 128 : (blk + 1) * 128],
                                        mt[:, cc, h * 512 : (h + 1) * 512],
                                        start=(cc == 0),
                                        stop=(cc == CC - 1),
                                    )
                                for cc in range(CC):
                                    nc.tensor.matmul(
                                        psS[:, h * 512 : (h + 1) * 512],
                                        ns[:, cc, blk * 128 : (blk + 1) * 128],
                                        ms[:, cc, h * 512 : (h + 1) * 512],
                                        start=(cc == 0),
                                        stop=(cc == CC - 1),
                                    )
                            eT = wpool.tile([128, N], F32, tag="eT")
                            eS = wpool.tile([128, N], F32, tag="eS")
                            nc.scalar.activation(
                                eT[:], psT[:], AF.Exp,
                                scale=1.0 / TEMPERATURE,
                                accum_out=zT[:, col : col + 1],
                            )
                            nc.scalar.activation(
                                eS[:], psS[:], AF.Exp,
                                scale=1.0 / TEMPERATURE,
                                accum_out=zS[:, col : col + 1],
                            )
                            junk1 = wpool.tile([128, N], F32, tag="junk1")
                            junk2 = wpool.tile([128, N], F32, tag="junk2")
                            nc.vector.scalar_tensor_tensor(
                                out=junk1[:], in0=eT[:], scalar=1.0, in1=psT[:],
                                op0=ALU.mult, op1=ALU.mult,
                                accum_out=r1[:, col : col + 1],
                            )
                            nc.vector.scalar_tensor_tensor(
                                out=junk2[:], in0=eT[:], scalar=1.0, in1=psS[:],
                                op0=ALU.mult, op1=ALU.mult,
                                accum_out=r2[:, col : col + 1],
                            )

                # ---------------- final combine ----------------
                ncols = NPAIR * NBLK
                recT = apool.tile([128, ncols], F32)
                nc.vector.reciprocal(recT[:], zT[:])
                rd = apool.tile([128, ncols], F32)
                nc.vector.tensor_sub(rd[:], r1[:], r2[:])
                kl1 = apool.tile([128, ncols], F32)
                nc.vector.scalar_tensor_tensor(
                    out=kl1[:], in0=rd[:], scalar=1.0 / TEMPERATURE, in1=recT[:],
                    op0=ALU.mult, op1=ALU.mult,
                )
                lnT = apool.tile([128, ncols], F32)
                nc.scalar.activation(lnT[:], zT[:], AF.Ln)
                lnS = apool.tile([128, ncols], F32)
                nc.scalar.activation(lnS[:], zS[:], AF.Ln)
                kl2 = apool.tile([128, ncols], F32)
                nc.vector.tensor_sub(kl2[:], kl1[:], lnT[:])
                kl3 = apool.tile([128, ncols], F32)
                nc.vector.tensor_add(kl3[:], kl2[:], lnS[:])
                klsum = apool.tile([128, 1], F32)
                nc.vector.reduce_sum(klsum[:], kl3[:], axis=mybir.AxisListType.X)
                scaled = apool.tile([128, 1], F32)
                nc.scalar.mul(scaled[:], klsum[:], 1.0 / (N * B * B))
                with tc.tile_pool(name="tot_ps", bufs=1, space="PSUM") as tpool:
                    tot_ps = tpool.tile([1, 1], F32)
                    nc.tensor.matmul(tot_ps[:], scaled[:], ones_col[:], start=True, stop=True)
                    outsb = apool.tile([1, 1], F32)
                    nc.scalar.copy(outsb[:], tot_ps[:])
                    nc.sync.dma_start(out_partial[:], outsb[:])

    return nc


_NC_CACHE = None


def _get_nc():
    global _NC_CACHE
    if _NC_CACHE is None:
        _NC_CACHE = legalize_waits(build_bass())
    return _NC_CACHE


def make_in_maps(feat_S, feat_T):
    feat_S = np.asarray(feat_S, dtype=np.float32)
    feat_T = np.asarray(feat_T, dtype=np.float32)
    in_maps = []
    for c in range(NCORES):
        img = feat_S[c] if c < B else feat_T[c - B]
        prs = PAIRS_PER_CORE[c]
        selv = [prs[0][0], prs[0][1], prs[1][0], prs[1][1]]
        selv = selv + [x + B for x in selv]
        in_maps.append(
            {
                "img": np.ascontiguousarray(img),
                "sel": np.asarray(selv, dtype=np.int32).reshape(1, 8),
            }
        )
    return in_maps


def run(feat_S, feat_T, **run_kwargs):
    nc = _get_nc()
    in_maps = make_in_maps(feat_S, feat_T)
    res = run_bass_kernel_spmd(nc, in_maps, core_ids=list(range(NCORES)), **run_kwargs)
    total = np.float32(0.0)
    for r in res.results:
        total += np.float32(r["out_partial"].reshape(()))
    return np.asarray(total, dtype=np.float32), res


def kernel(**inputs):
    out, _ = run(inputs["feat_S"], inputs["feat_T"])
    return out


# revision 12
# speedup vs baseline: 7.1123x; 7.1123x over previous
"""Trainium2 Bass kernel for CriterionMiniBatchCrossImagePair.

Computes: prep = L2norm_C(avgpool4x4(x)) per image -> all BxB pairwise
[N,N] similarity maps for S and T -> KL(softmax_T || softmax_S) batchmean.

Sharding: 8 cores. Each core preps ONE of the 8 images (4 S + 4 T),
AllGathers the prepped bf16 features [256,1024], then computes 2 of the
16 (i,j) pairs (transpose-pairing so each core touches only 2 image
indices). Scalar partials are summed on the host.

Math used per row-block (row softmax over m):
  KL_row = sum_m p_t*(log p_t - log p_s)
         = (1/T) * (sum_m eT*rawT - sum_m eT*rawS) / Z_T - ln Z_T + ln Z_S
with eX = exp(rawX/T), Z_X = sum_m eX. No max-subtraction needed:
raw in [-1,1] so raw/T in [-10,10].
"""

import numpy as np

import concourse.bass as bass
import concourse.mybir as mybir
import concourse.tile as tile
from concourse.bass_utils import run_bass_kernel_spmd

F32 = mybir.dt.float32
BF16 = mybir.dt.bfloat16
I32 = mybir.dt.int32
AF = mybir.ActivationFunctionType
ALU = mybir.AluOpType

TEMPERATURE = 0.1
B, C, H, W = 4, 256, 128, 128
PATCH = 4
PH, PW = H // PATCH, W // PATCH  # 32 x 32
N = PH * PW  # 1024
NCORES = 8
CC = C // 128  # channel chunks of 128
FB = 4  # h-row chunks per channel chunk (32 h rows each)
HROWS = H // FB  # 32
NBLK = N // 128  # 8 row blocks per pair
NPAIR = 2  # pairs per core

# core -> [(n0, m0), (n1, m1)] image-index pairs (covers all 16 (i,j))
PAIRS_PER_CORE = [
    [(0, 0), (1, 1)],
    [(2, 2), (3, 3)],
    [(0, 1), (1, 0)],
    [(2, 3), (3, 2)],
    [(0, 2), (2, 0)],
    [(1, 3), (3, 1)],
    [(0, 3), (3, 0)],
    [(1, 2), (2, 1)],
]


def legalize_waits(nc):
    """Split multi-wait instructions into single-wait NoOps.

    The walrus build in this environment encodes at most one sync-wait per
    instruction (and none on register-offset pseudo DMAs): anything more dies
    in codegen with "Too many sync wait commands". Semantically, hoisting a
    wait onto a NoOp immediately before the instruction on the same engine
    stream is identical (both block the engine's sequencer).
    """
    n_id = 0
    for f in nc.m.functions:
        for b in f.blocks:
            lst = b.instructions
            out = []
            changed = False
            for ins in lst:
                si = ins.sync_info
                waits = list(si.on_wait) if si and si.on_wait else []
                keep = 0 if isinstance(ins, mybir.InstDMACopy) else 1
                if len(waits) > keep:
                    moved, kept = waits[: len(waits) - keep], waits[len(waits) - keep :]
                    for w in moved:
                        nop = mybir.InstNoOp(name=f"waitnop_{n_id}")
                        n_id += 1
                        nop.engine = ins.engine
                        nop.sync_info = mybir.SyncInfo(on_wait=[w], on_update=[])
                        out.append(nop)
                    ins.sync_info = mybir.SyncInfo(
                        on_wait=kept, on_update=list(si.on_update)
                    )
                    changed = True
                out.append(ins)
            if changed:
                b.instructions = out
    return nc


def build_bass():
    nc = bass.Bass(num_devices=NCORES)

    img = nc.declare_dram_parameter("img", [C, H, W], F32, isOutput=False)
    sel = nc.declare_dram_parameter("sel", [1, 8], I32, isOutput=False)
    out_partial = nc.declare_dram_parameter("out_partial", [1, 1], F32, isOutput=True)

    with tile.TileContext(nc) as tc:
        with (
            tc.tile_pool(name="dram", bufs=1, space="DRAM") as dpool,
            tc.tile_pool(name="consts", bufs=1) as cpool,
        ):
            ag_in = dpool.tile([128, CC, N], BF16, name="ag_in")
            ag_out = dpool.tile(
                [NCORES, 128, CC, N], BF16, addr_space="Shared", name="ag_out"
            )
            ones_col = cpool.tile([128, 1], F32)
            nc.vector.memset(ones_col[:], 1.0)
            ones_row = cpool.tile([1, 128], F32)
            nc.vector.memset(ones_row[:], 1.0)

            # ---------------- Stage A: prep own image ----------------
            with (
                tc.tile_pool(name="prep", bufs=3) as ppool,
                tc.tile_pool(name="prep_ps", bufs=1, space="PSUM") as pspool,
                tc.tile_pool(name="prep_keep", bufs=1) as kpool,
            ):
                u = kpool.tile([128, CC, PH, PW], F32)  # pooled (unnormalized)
                ss_ps = pspool.tile([1, N], F32)  # sum_c u^2
                for cc in range(CC):
                    for fb in range(FB):
                        raw = ppool.tile([128, HROWS, W], F32, tag="raw")
                        nc.sync.dma_start(
                            raw[:],
                            img[cc * 128 : (cc + 1) * 128, fb * HROWS : (fb + 1) * HROWS, :],
                        )
                        wp1 = ppool.tile([128, HROWS, PW], F32, tag="wp1")
                        wp2 = ppool.tile([128, HROWS, PW], F32, tag="wp2")
                        wp = ppool.tile([128, HROWS, PW], F32, tag="wp")
                        nc.vector.tensor_add(wp1[:], raw[:, :, 0::4], raw[:, :, 1::4])
                        nc.vector.tensor_add(wp2[:], raw[:, :, 2::4], raw[:, :, 3::4])
                        nc.vector.tensor_add(wp[:], wp1[:], wp2[:])
                        hp1 = ppool.tile([128, HROWS // 4, PW], F32, tag="hp1")
                        hp2 = ppool.tile([128, HROWS // 4, PW], F32, tag="hp2")
                        nc.vector.tensor_add(hp1[:], wp[:, 0::4, :], wp[:, 1::4, :])
                        nc.vector.tensor_add(hp2[:], wp[:, 2::4, :], wp[:, 3::4, :])
                        nc.vector.tensor_add(
                            u[:, cc, fb * (HROWS // 4) : (fb + 1) * (HROWS // 4), :],
                            hp1[:],
                            hp2[:],
                        )

                # sum over channels of u^2 (via ones-matmul), both c-chunks
                for cc in range(CC):
                    sq = ppool.tile([128, N], F32, tag="sq")
                    ucc = u[:, cc].rearrange("p a b -> p (a b)")
                    nc.vector.tensor_mul(sq[:], ucc, ucc)
                    for h in range(2):
                        nc.tensor.matmul(
                            ss_ps[:, h * 512 : (h + 1) * 512],
                            ones_col[:],
                            sq[:, h * 512 : (h + 1) * 512],
                            start=(cc == 0),
                            stop=(cc == CC - 1),
                        )

                # inv = ss^(-1/2) = exp(-0.5*ln(ss)) on 1 partition
                lnss = kpool.tile([1, N], F32)
                nc.scalar.activation(lnss[:], ss_ps[:], AF.Ln)
                inv = kpool.tile([1, N], F32)
                nc.scalar.activation(inv[:], lnss[:], AF.Exp, scale=-0.5)

                # broadcast inv to 128 partitions via ones-matmul
                inv_b = pspool.tile([128, N], F32)
                for h in range(2):
                    nc.tensor.matmul(
                        inv_b[:, h * 512 : (h + 1) * 512],
                        ones_row[:],
                        inv[:, h * 512 : (h + 1) * 512],
                        start=True,
                        stop=True,
                    )

                feat = kpool.tile([128, CC, N], BF16)
                for cc in range(CC):
                    ucc = u[:, cc].rearrange("p a b -> p (a b)")
                    nc.vector.tensor_mul(feat[:, cc], ucc, inv_b[:])
                    nc.sync.dma_start(ag_in[:, cc, :], feat[:, cc])

            # ---------------- AllGather prepped features ----------------
            nc.gpsimd.collective_compute(
                "AllGather",
                ALU.bypass,
                replica_groups=[list(range(NCORES))],
                ins=[ag_in.opt()],
                outs=[ag_out.opt()],
            )

            # ---------------- Stage B: 2 pairs of similarity maps ----------------
            with (
                tc.tile_pool(name="slots", bufs=1) as spool,
                tc.tile_pool(name="acc", bufs=1) as apool,
                tc.tile_pool(name="work", bufs=2) as wpool,
            ):
                sel_sb = apool.tile([1, 8], I32)
                nc.sync.dma_start(sel_sb[:], sel[:])

                # slot order: NS0, MS0, NS1, MS1, NT0, MT0, NT1, MT1
                slots = []
                for s in range(8):
                    slot = spool.tile([128, CC, N], BF16, name=f"slot{s}")
                    v = nc.sync.value_load(sel_sb[0:1, s : s + 1])
                    nc.sync.dma_start(slot[:], ag_out[bass.ds(v, 1)].squeeze(0))
                    slots.append(slot)

                zT = apool.tile([128, NPAIR * NBLK], F32)
                zS = apool.tile([128, NPAIR * NBLK], F32)
                r1 = apool.tile([128, NPAIR * NBLK], F32)
                r2 = apool.tile([128, NPAIR * NBLK], F32)

                with tc.tile_pool(name="sims_ps", bufs=2, space="PSUM") as simspool:
                    for p in range(NPAIR):
                        ns, ms = slots[2 * p], slots[2 * p + 1]
                        nt, mt = slots[4 + 2 * p], slots[4 + 2 * p + 1]
                        for blk in range(NBLK):
                            col = p * NBLK + blk
                            psS = simspool.tile([128, N], F32, tag="psS")
                            psT = simspool.tile([128, N], F32, tag="psT")
                            for h in range(2):
                                for cc in range(CC):
                                    nc.tensor.matmul(
                                        psT[:, h * 512 : (h + 1) * 512],
                                        nt[:, cc, blk * 128 : (blk + 1) * 128],
                                        mt[:, cc, h * 512 : (h + 1) * 512],
                                        start=(cc == 0),
                                        stop=(cc == CC - 1),
                                    )
                                for cc in range(CC):
                                    nc.tensor.matmul(
                                        psS[:, h * 512 : (h + 1) * 512],
                                        ns[:, cc, blk * 128 : (blk + 1) * 128],
                                        ms[:, cc, h * 512 : (h + 1) * 512],
                                        start=(cc == 0),
                                        stop=(cc == CC - 1),
                                    )
                            eT = wpool.tile([128, N], F32, tag="eT")
                            eS = wpool.tile([128, N], F32, tag="eS")
                            nc.scalar.activation(
                                eT[:], psT[:], AF.Exp,
                                scale=1.0 / TEMPERATURE,
                                accum_out=zT[:, col : col + 1],
                            )
                            nc.scalar.activation(
                                eS[:], psS[:], AF.Exp,
                                scale=1.0 / TEMPERATURE,
                                accum_out=zS[:, col : col + 1],
                            )
                            junk1 = wpool.tile([128, N], F32, tag="junk1")
                            junk2 = wpool.tile([128, N], F32, tag="junk2")
                            nc.vector.scalar_tensor_tensor(
                                out=junk1[:], in0=eT[:], scalar=1.0, in1=psT[:],
                                op0=ALU.mult, op1=ALU.mult,
                                accum_out=r1[:, col : col + 1],
                            )
                            nc.vector.scalar_tensor_tensor(
                                out=junk2[:], in0=eT[:], scalar=1.0, in1=psS[:],
                                op0=ALU.mult, op1=ALU.mult,
                                accum_out=r2[:, col : col + 1],
                            )

                # ---------------- final combine ----------------
                ncols = NPAIR * NBLK
                recT = apool.tile([128, ncols], F32)
                nc.vector.reciprocal(recT[:], zT[:])
                rd = apool.tile([128, ncols], F32)
                nc.vector.tensor_sub(rd[:], r1[:], r2[:])
                kl1 = apool.tile([128, ncols], F32)
                nc.vector.scalar_tensor_tensor(
                    out=kl1[:], in0=rd[:], scalar=1.0 / TEMPERATURE, in1=recT[:],
                    op0=ALU.mult, op1=ALU.mult,
                )
                lnT = apool.tile([128, ncols], F32)
                nc.scalar.activation(lnT[:], zT[:], AF.Ln)
                lnS = apool.tile([128, ncols], F32)
                nc.scalar.activation(lnS[:], zS[:], AF.Ln)
                kl2 = apool.tile([128, ncols], F32)
                nc.vector.tensor_sub(kl2[:], kl1[:], lnT[:])
                kl3 = apool.tile([128, ncols], F32)
                nc.vector.tensor_add(kl3[:], kl2[:], lnS[:])
                klsum = apool.tile([128, 1], F32)
                nc.vector.reduce_sum(klsum[:], kl3[:], axis=mybir.AxisListType.X)
                scaled = apool.tile([128, 1], F32)
                nc.scalar.mul(scaled[:], klsum[:], 1.0 / (N * B * B))
                with tc.tile_pool(name="tot_ps", bufs=1, space="PSUM") as tpool:
                    tot_ps = tpool.tile([1, 1], F32)
                    nc.tensor.matmul(tot_ps[:], scaled[:], ones_col[:], start=True, stop=True)
                    outsb = apool.tile([1, 1], F32)
                    nc.scalar.copy(outsb[:], tot_ps[:])
                    nc.sync.dma_start(out_partial[:], outsb[:])

    return nc


_NC_CACHE = None


def _get_nc():
    global _NC_CACHE
    if _NC_CACHE is None:
        _NC_CACHE = legalize_waits(build_bass())
    return _NC_CACHE


def make_in_maps(feat_S, feat_T):
    feat_S = np.asarray(feat_S, dtype=np.float32)
    feat_T = np.asarray(feat_T, dtype=np.float32)
    in_maps = []
    for c in range(NCORES):
        img = feat_S[c] if c < B else feat_T[c - B]
        prs = PAIRS_PER_CORE[c]
        selv = [prs[0][0], prs[0][1], prs[1][0], prs[1][1]]
        selv = selv + [x + B for x in selv]
        in_maps.append(
            {
                "img": np.ascontiguousarray(img),
                "sel": np.asarray(selv, dtype=np.int32).reshape(1, 8),
            }
        )
    return in_maps


def run(feat_S, feat_T, **run_kwargs):
    nc = _get_nc()
    in_maps = make_in_maps(feat_S, feat_T)
    res = run_bass_kernel_spmd(nc, in_maps, core_ids=list(range(NCORES)), **run_kwargs)
    total = np.float32(0.0)
    for r in res.results:
        total += np.float32(r["out_partial"].reshape(()))
    return np.asarray(total, dtype=np.float32), res


def kernel(**inputs):
    out, _ = run(inputs["feat_S"], inputs["feat_T"])
    return out


# revision 13
# speedup vs baseline: 7.4763x; 1.0512x over previous
"""Trainium2 Bass kernel for CriterionMiniBatchCrossImagePair.

Computes: prep = L2norm_C(avgpool4x4(x)) per image -> all BxB pairwise
[N,N] similarity maps for S and T -> KL(softmax_T || softmax_S) batchmean.

Sharding: 8 cores. Each core preps ONE of the 8 images (4 S + 4 T),
AllGathers the prepped bf16 features [256,1024], then computes 2 of the
16 (i,j) pairs (transpose-pairing so each core touches only 2 image
indices). Scalar partials are summed on the host.

Math used per row-block (row softmax over m):
  KL_row = sum_m p_t*(log p_t - log p_s)
         = (1/T) * (sum_m eT*rawT - sum_m eT*rawS) / Z_T - ln Z_T + ln Z_S
with eX = exp(rawX/T), Z_X = sum_m eX. No max-subtraction needed:
raw in [-1,1] so raw/T in [-10,10].
"""

import numpy as np

import concourse.bass as bass
import concourse.mybir as mybir
import concourse.tile as tile
from concourse.bass_utils import run_bass_kernel_spmd

F32 = mybir.dt.float32
BF16 = mybir.dt.bfloat16
I32 = mybir.dt.int32
AF = mybir.ActivationFunctionType
ALU = mybir.AluOpType

TEMPERATURE = 0.1
B, C, H, W = 4, 256, 128, 128
PATCH = 4
PH, PW = H // PATCH, W // PATCH  # 32 x 32
N = PH * PW  # 1024
NCORES = 8
CC = C // 128  # channel chunks of 128
FB = 4  # h-row chunks per channel chunk (32 h rows each)
HROWS = H // FB  # 32
NBLK = N // 128  # 8 row blocks per pair
NPAIR = 2  # pairs per core

# core -> [(n0, m0), (n1, m1)] image-index pairs (covers all 16 (i,j))
PAIRS_PER_CORE = [
    [(0, 0), (1, 1)],
    [(2, 2), (3, 3)],
    [(0, 1), (1, 0)],
    [(2, 3), (3, 2)],
    [(0, 2), (2, 0)],
    [(1, 3), (3, 1)],
    [(0, 3), (3, 0)],
    [(1, 2), (2, 1)],
]


def legalize_waits(nc):
    """Split multi-wait instructions into single-wait NoOps.

    The walrus build in this environment encodes at most one sync-wait per
    instruction (and none on register-offset pseudo DMAs): anything more dies
    in codegen with "Too many sync wait commands". Semantically, hoisting a
    wait onto a NoOp immediately before the instruction on the same engine
    stream is identical (both block the engine's sequencer).
    """
    n_id = 0
    for f in nc.m.functions:
        for b in f.blocks:
            lst = b.instructions
            out = []
            changed = False
            for ins in lst:
                si = ins.sync_info
                waits = list(si.on_wait) if si and si.on_wait else []
                keep = 0 if isinstance(ins, mybir.InstDMACopy) else 1
                if len(waits) > keep:
                    moved, kept = waits[: len(waits) - keep], waits[len(waits) - keep :]
                    for w in moved:
                        nop = mybir.InstNoOp(name=f"waitnop_{n_id}")
                        n_id += 1
                        nop.engine = ins.engine
                        nop.sync_info = mybir.SyncInfo(on_wait=[w], on_update=[])
                        out.append(nop)
                    ins.sync_info = mybir.SyncInfo(
                        on_wait=kept, on_update=list(si.on_update)
                    )
                    changed = True
                out.append(ins)
            if changed:
                b.instructions = out
    return nc


def build_bass():
    nc = bass.Bass(num_devices=NCORES)

    img = nc.declare_dram_parameter("img", [C, H, W], F32, isOutput=False)
    sel = nc.declare_dram_parameter("sel", [1, 8], I32, isOutput=False)
    out_partial = nc.declare_dram_parameter("out_partial", [1, 1], F32, isOutput=True)

    with tile.TileContext(nc) as tc:
        with (
            tc.tile_pool(name="dram", bufs=1, space="DRAM") as dpool,
            tc.tile_pool(name="consts", bufs=1) as cpool,
        ):
            ag_in_h = [
                dpool.tile([128, CC, N // 2], BF16, name=f"ag_in{h}")
                for h in range(2)
            ]
            ag_out_h = [
                dpool.tile(
                    [NCORES, 128, CC, N // 2], BF16, addr_space="Shared",
                    name=f"ag_out{h}",
                )
                for h in range(2)
            ]
            ones_col = cpool.tile([128, 1], F32)
            nc.vector.memset(ones_col[:], 1.0)
            ones_row = cpool.tile([1, 128], F32)
            nc.vector.memset(ones_row[:], 1.0)

            # ---------------- Stage A: prep own image ----------------
            with (
                tc.tile_pool(name="prep", bufs=3) as ppool,
                tc.tile_pool(name="prep_ps", bufs=1, space="PSUM") as pspool,
                tc.tile_pool(name="prep_keep", bufs=1) as kpool,
            ):
                u = kpool.tile([128, CC, PH, PW], F32)  # pooled (unnormalized)
                ss_ps = pspool.tile([1, N], F32)  # sum_c u^2
                # half-major: columns n in [half*512,(half+1)*512) finish
                # (pool+norm) and AllGather while the other half still preps
                for half in range(2):
                    for cc in range(CC):
                        for fb in (2 * half, 2 * half + 1):
                            raw = ppool.tile([128, HROWS, W], F32, tag="raw")
                            nc.sync.dma_start(
                                raw[:],
                                img[cc * 128 : (cc + 1) * 128, fb * HROWS : (fb + 1) * HROWS, :],
                            )
                            wp1 = ppool.tile([128, HROWS, PW], F32, tag="wp1")
                            wp2 = ppool.tile([128, HROWS, PW], F32, tag="wp2")
                            wp = ppool.tile([128, HROWS, PW], F32, tag="wp")
                            nc.vector.tensor_add(wp1[:], raw[:, :, 0::4], raw[:, :, 1::4])
                            nc.vector.tensor_add(wp2[:], raw[:, :, 2::4], raw[:, :, 3::4])
                            nc.vector.tensor_add(wp[:], wp1[:], wp2[:])
                            hp1 = ppool.tile([128, HROWS // 4, PW], F32, tag="hp1")
                            hp2 = ppool.tile([128, HROWS // 4, PW], F32, tag="hp2")
                            nc.vector.tensor_add(hp1[:], wp[:, 0::4, :], wp[:, 1::4, :])
                            nc.vector.tensor_add(hp2[:], wp[:, 2::4, :], wp[:, 3::4, :])
                            nc.vector.tensor_add(
                                u[:, cc, fb * (HROWS // 4) : (fb + 1) * (HROWS // 4), :],
                                hp1[:],
                                hp2[:],
                            )
                    # norm of this half's 512 columns
                    ph0 = half * (PH // 2)
                    for cc in range(CC):
                        sq = ppool.tile([128, N // 2], F32, tag="sq")
                        uh = u[:, cc, ph0 : ph0 + PH // 2, :].rearrange("p a b -> p (a b)")
                        nc.vector.tensor_mul(sq[:], uh, uh)
                        nc.tensor.matmul(
                            ss_ps[:, half * 512 : (half + 1) * 512],
                            ones_col[:],
                            sq[:],
                            start=(cc == 0),
                            stop=(cc == CC - 1),
                        )
                    lnss = kpool.tile([1, N // 2], F32, name=f"lnss{half}")
                    nc.scalar.activation(lnss[:], ss_ps[:, half * 512 : (half + 1) * 512], AF.Ln)
                    inv = kpool.tile([1, N // 2], F32, name=f"inv{half}")
                    nc.scalar.activation(inv[:], lnss[:], AF.Exp, scale=-0.5)
                    inv_b = pspool.tile([128, N // 2], F32, tag="inv_b", bufs=2)
                    nc.tensor.matmul(inv_b[:], ones_row[:], inv[:], start=True, stop=True)
                    for cc in range(CC):
                        feth = ppool.tile([128, N // 2], BF16, tag="feth")
                        uh = u[:, cc, ph0 : ph0 + PH // 2, :].rearrange("p a b -> p (a b)")
                        nc.vector.tensor_mul(feth[:], uh, inv_b[:])
                        nc.sync.dma_start(ag_in_h[half][:, cc, :], feth[:])
                    nc.gpsimd.collective_compute(
                        "AllGather",
                        ALU.bypass,
                        replica_groups=[list(range(NCORES))],
                        ins=[ag_in_h[half].opt()],
                        outs=[ag_out_h[half].opt()],
                    )

            # ---------------- Stage B: 2 pairs of similarity maps ----------------
            with (
                tc.tile_pool(name="slots", bufs=1) as spool,
                tc.tile_pool(name="acc", bufs=1) as apool,
                tc.tile_pool(name="work", bufs=2) as wpool,
            ):
                sel_sb = apool.tile([1, 8], I32)
                nc.sync.dma_start(sel_sb[:], sel[:])

                # slot order: NS0, MS0, NS1, MS1, NT0, MT0, NT1, MT1
                # slots[s][half] is a [128, CC, 512] tile of columns half*512+
                slots = []
                for s in range(8):
                    v = nc.sync.value_load(sel_sb[0:1, s : s + 1])
                    sh = []
                    for half in range(2):
                        t = spool.tile([128, CC, N // 2], BF16, name=f"slot{s}h{half}")
                        nc.sync.dma_start(t[:], ag_out_h[half][bass.ds(v, 1)].squeeze(0))
                        sh.append(t)
                    slots.append(sh)

                zT = apool.tile([128, NPAIR * NBLK], F32)
                zS = apool.tile([128, NPAIR * NBLK], F32)
                r1 = apool.tile([128, NPAIR * NBLK], F32)
                r2 = apool.tile([128, NPAIR * NBLK], F32)

                with tc.tile_pool(name="sims_ps", bufs=2, space="PSUM") as simspool:
                    for p in range(NPAIR):
                        ns, ms = slots[2 * p], slots[2 * p + 1]
                        nt, mt = slots[4 + 2 * p], slots[4 + 2 * p + 1]
                        for blk in range(NBLK):
                            col = p * NBLK + blk
                            nh, nloc = blk // 4, (blk % 4) * 128
                            psS = simspool.tile([128, N], F32, tag="psS")
                            psT = simspool.tile([128, N], F32, tag="psT")
                            for h in range(2):
                                for cc in range(CC):
                                    nc.tensor.matmul(
                                        psT[:, h * 512 : (h + 1) * 512],
                                        nt[nh][:, cc, nloc : nloc + 128],
                                        mt[h][:, cc, :],
                                        start=(cc == 0),
                                        stop=(cc == CC - 1),
                                    )
                                for cc in range(CC):
                                    nc.tensor.matmul(
                                        psS[:, h * 512 : (h + 1) * 512],
                                        ns[nh][:, cc, nloc : nloc + 128],
                                        ms[h][:, cc, :],
                                        start=(cc == 0),
                                        stop=(cc == CC - 1),
                                    )
                            eT = wpool.tile([128, N], F32, tag="eT")
                            eS = wpool.tile([128, N], F32, tag="eS")
                            nc.scalar.activation(
                                eT[:], psT[:], AF.Exp,
                                scale=1.0 / TEMPERATURE,
                                accum_out=zT[:, col : col + 1],
                            )
                            nc.scalar.activation(
                                eS[:], psS[:], AF.Exp,
                                scale=1.0 / TEMPERATURE,
                                accum_out=zS[:, col : col + 1],
                            )
                            junk1 = wpool.tile([128, N], F32, tag="junk1")
                            junk2 = wpool.tile([128, N], F32, tag="junk2")
                            nc.vector.scalar_tensor_tensor(
                                out=junk1[:], in0=eT[:], scalar=1.0, in1=psT[:],
                                op0=ALU.mult, op1=ALU.mult,
                                accum_out=r1[:, col : col + 1],
                            )
                            nc.vector.scalar_tensor_tensor(
                                out=junk2[:], in0=eT[:], scalar=1.0, in1=psS[:],
                                op0=ALU.mult, op1=ALU.mult,
                                accum_out=r2[:, col : col + 1],
                            )

                # ---------------- final combine ----------------
                ncols = NPAIR * NBLK
                recT = apool.tile([128, ncols], F32)
                nc.vector.reciprocal(recT[:], zT[:])
                rd = apool.tile([128, ncols], F32)
                nc.vector.tensor_sub(rd[:], r1[:], r2[:])
                kl1 = apool.tile([128, ncols], F32)
                nc.vector.scalar_tensor_tensor(
                    out=kl1[:], in0=rd[:], scalar=1.0 / TEMPERATURE, in1=recT[:],
                    op0=ALU.mult, op1=ALU.mult,
                )
                lnT = apool.tile([128, ncols], F32)
                nc.scalar.activation(lnT[:], zT[:], AF.Ln)
                lnS = apool.tile([128, ncols], F32)
                nc.scalar.activation(lnS[:], zS[:], AF.Ln)
                kl2 = apool.tile([128, ncols], F32)
                nc.vector.tensor_sub(kl2[:], kl1[:], lnT[:])
                kl3 = apool.tile([128, ncols], F32)
                nc.vector.tensor_add(kl3[:], kl2[:], lnS[:])
                klsum = apool.tile([128, 1], F32)
                nc.vector.reduce_sum(klsum[:], kl3[:], axis=mybir.AxisListType.X)
                scaled = apool.tile([128, 1], F32)
                nc.scalar.mul(scaled[:], klsum[:], 1.0 / (N * B * B))
                with tc.tile_pool(name="tot_ps", bufs=1, space="PSUM") as tpool:
                    tot_ps = tpool.tile([1, 1], F32)
                    nc.tensor.matmul(tot_ps[:], scaled[:], ones_col[:], start=True, stop=True)
                    outsb = apool.tile([1, 1], F32)
                    nc.scalar.copy(outsb[:], tot_ps[:])
                    nc.sync.dma_start(out_partial[:], outsb[:])

    return nc


_NC_CACHE = None


def _get_nc():
    global _NC_CACHE
    if _NC_CACHE is None:
        _NC_CACHE = legalize_waits(build_bass())
    return _NC_CACHE


def make_in_maps(feat_S, feat_T):
    feat_S = np.asarray(feat_S, dtype=np.float32)
    feat_T = np.asarray(feat_T, dtype=np.float32)
    in_maps = []
    for c in range(NCORES):
        img = feat_S[c] if c < B else feat_T[c - B]
        prs = PAIRS_PER_CORE[c]
        selv = [prs[0][0], prs[0][1], prs[1][0], prs[1][1]]
        selv = selv + [x + B for x in selv]
        in_maps.append(
            {
                "img": np.ascontiguousarray(img),
                "sel": np.asarray(selv, dtype=np.int32).reshape(1, 8),
            }
        )
    return in_maps


def run(feat_S, feat_T, **run_kwargs):
    nc = _get_nc()
    in_maps = make_in_maps(feat_S, feat_T)
    res = run_bass_kernel_spmd(nc, in_maps, core_ids=list(range(NCORES)), **run_kwargs)
    total = np.float32(0.0)
    for r in res.results:
        total += np.float32(r["out_partial"].reshape(()))
    return np.asarray(total, dtype=np.float32), res


def kernel(**inputs):
    out, _ = run(inputs["feat_S"], inputs["feat_T"])
    return out
